# revision 10
# baseline (speedup 1.0000x reference)
"""Trainium2 Bass kernel for NeighborAggregation.

Math: for x of shape (b, k=1024, c=512) viewed as a 32x32 grid over k,
the reference computes y[cell t] = s(t) * 8^(t-1024) where s is a sum of 4
circularly-shifted neighbors minus 4x, and returns concat(x, y) on the c axis.
8^(t-1024) underflows to exactly 0.0 in fp32 for t <= 974, so y is nonzero
only for the last 49 k-rows (t = 975..1023), whose neighbor cells all live in
grid rows {0, 28..31} = flat cells [0..31] and [896..1023].

Kernel strategy (pure data parallel, batch 64 -> 8 cores x 8 examples):
  1. The bulk x -> out[:, :, 0:512] copy (16.78 MB/core) dominates: every
     byte crosses one of the 16 SDMA engines once at ~25.6 GB/s/engine.
     It is split into two b-contiguous halves, one per HWDGE ring (SP via
     nc.sync + ACT via nc.scalar). Each half's source AP collapses to one
     flat dim (major % 16 == 0), which is required for the descriptor
     generator to spray descriptors over all 16 engines -- non-16-divisible
     major dims fan over only ceil(major/8) engines starting at engine 0.
  2. The 49 nonzero y rows per example come from a sparse fp32 matmul pair
     (cells 896..1023 and 0..31 on partitions) with the 8^(t-1024) factors
     folded into the weights. Loads are per-example so matmuls pipeline
     behind the edge-row loads instead of waiting for one big transfer.
  3. Each example's y store is split 48 rows + 1 row: the 48-row store has
     a 16-divisible major dim and sprays all 16 engines (3 descriptors
     each) instead of piling 49 descriptors on engines 0-6.
  4. Edge rows and weights are uploaded pre-transposed/contiguous so loads
     are few large line-rate descriptors.
  5. The zero region of y is never written: ExternalOutput buffers are
     pre-zeroed by the runner.
"""

from contextlib import ExitStack

import numpy as np

_B_FULL, _K, _C = 64, 1024, 512
_NCORES = 8
_B = _B_FULL // _NCORES  # examples per core
_N = 32
_HI = 896  # first cell of grid rows 28..31
_NNZ = 49  # cells 975..1023 have nonzero factor
_Y0 = _K - _NNZ  # 975
_BSPLIT = 4  # copy: examples 0:4 on the sync ring; rest on scalar

_cached = {}


def _weights():
    """W1 (128, 49) over cells 896..1023 and W2 (32, 49) over cells 0..31.

    Column o corresponds to output cell k = 975 + o; entries are the neighbor
    coefficients scaled by factor[k] = 8^(k-1024) (exact in fp32).
    """
    t = np.arange(_K)
    factor = (np.float64(2.0) ** (3.0 * (t - _K))).astype(np.float32)
    w1 = np.zeros((128, _NNZ), np.float32)
    w2 = np.zeros((_N, _NNZ), np.float32)
    for o in range(_NNZ):
        k = _Y0 + o
        i, j = divmod(k, _N)
        f = factor[k]
        i1, i2 = (i + 1) % _N, (i - 2) % _N
        jp, jm = (j + 1) % _N, (j - 2) % _N
        for r, q in [(i1, jp), (i1, jm), (i2, jp), (i2, jm)]:
            cell = _N * r + q
            if cell >= _HI:
                w1[cell - _HI, o] += f
            else:
                w2[cell, o] += f
        w1[k - _HI, o] += np.float32(-4.0) * f
    return w1, w2


def _build_nc():
    import concourse.bacc as bacc
    import concourse.mybir as mybir
    import concourse.tile as tile

    nc = bacc.Bacc("TRN2", debug=False, num_devices=_NCORES)
    f32 = mybir.dt.float32
    x_ap = nc.dram_tensor("x", (_B, _K, _C), f32, kind="ExternalInput").ap()
    xe1_ap = nc.dram_tensor("xe1", (_B, 128, _C), f32, kind="ExternalInput").ap()
    xe2_ap = nc.dram_tensor("xe2", (_B, _N, _C), f32, kind="ExternalInput").ap()
    w1_ap = nc.dram_tensor("w1", (128, _NNZ), f32, kind="ExternalInput").ap()
    w2_ap = nc.dram_tensor("w2", (_N, _NNZ), f32, kind="ExternalInput").ap()
    out_ap = nc.dram_tensor("out", (_B, _K, 2 * _C), f32, kind="ExternalOutput").ap()

    with tile.TileContext(nc) as tc, ExitStack() as ctx:
        pool = ctx.enter_context(tc.tile_pool(name="sbuf", bufs=1))
        psum_pool = ctx.enter_context(tc.tile_pool(name="psum", bufs=4, space="PSUM"))

        # Sync-ring half of the bulk copy goes first in that ring's FIFO.
        nc.sync.dma_start(
            out=out_ap[0:_BSPLIT, :, 0:_C], in_=x_ap[0:_BSPLIT, :, :]
        )

        # Scalar ring: weight + edge loads first, then its copy half.
        w1 = pool.tile([128, _NNZ], f32, tag="w1")
        nc.scalar.dma_start(out=w1[:], in_=w1_ap)
        w2 = pool.tile([_N, _NNZ], f32, tag="w2")
        nc.scalar.dma_start(out=w2[:], in_=w2_ap)

        x1s = []
        x2s = []
        for b in range(_B):
            x1 = pool.tile([128, _C], f32, tag=f"x1_{b}", name=f"x1_{b}")
            nc.scalar.dma_start(out=x1[:], in_=xe1_ap[b])
            x1s.append(x1)
            x2 = pool.tile([_N, _C], f32, tag=f"x2_{b}", name=f"x2_{b}")
            nc.scalar.dma_start(out=x2[:], in_=xe2_ap[b])
            x2s.append(x2)

        nc.scalar.dma_start(
            out=out_ap[_BSPLIT:_B, :, 0:_C], in_=x_ap[_BSPLIT:_B, :, :]
        )

        for b in range(_B):
            ps = psum_pool.tile([_NNZ, _C], f32)
            nc.tensor.matmul(ps[:], w1[:], x1s[b][:], start=True, stop=False)
            nc.tensor.matmul(ps[:], w2[:], x2s[b][:], start=False, stop=True)
            y = pool.tile([_NNZ, _C], f32, tag=f"y_{b}", name=f"y_{b}")
            nc.vector.tensor_copy(y[:], ps[:])
            # 48-row store (major % 16 == 0 -> 16-way engine spray) on sync,
            # single leftover row on scalar.
            nc.sync.dma_start(
                out=out_ap[b, _Y0 : _Y0 + 48, _C : 2 * _C], in_=y[0:48, :]
            )
            nc.scalar.dma_start(
                out=out_ap[b, _Y0 + 48 : _K, _C : 2 * _C], in_=y[48:_NNZ, :]
            )

    nc.compile()
    return nc


def _get_nc():
    if "nc" not in _cached:
        _cached["nc"] = _build_nc()
    return _cached["nc"]


def _in_maps(x):
    w1, w2 = _weights()
    maps = []
    for i in range(_NCORES):
        xs = np.ascontiguousarray(x[i * _B : (i + 1) * _B])
        maps.append(
            {
                "x": xs,
                "xe1": np.ascontiguousarray(xs[:, _HI:_K, :]),
                "xe2": np.ascontiguousarray(xs[:, 0:_N, :]),
                "w1": w1,
                "w2": w2,
            }
        )
    return maps


def kernel(x):
    from concourse.bass_utils import run_bass_kernel_spmd

    x = np.asarray(x, dtype=np.float32)
    assert x.shape == (_B_FULL, _K, _C), x.shape
    nc = _get_nc()
    res = run_bass_kernel_spmd(nc, _in_maps(x), list(range(_NCORES)))
    return np.concatenate([r["out"] for r in res.results], axis=0)
